# revision 11
# baseline (speedup 1.0000x reference)
"""Trainium2 Bass kernel for nn_DCTLayer: per-8x8-block 2D DCT-like transform.

Math: reference computes, per 8x8 block X of the 256x256 image,
    out_block[y, v] = sum_x A[v, x] * X[x, y],   where A = D @ D
(D = 8x8 DCT basis). out_block = (A @ X)^T.

Kernel strategy (per core, pure data parallel over batch):
  - Load 128 consecutive image rows (a "strip") into SBUF naturally:
    partition p = strip row = (G, x) [G = row-block 0..15, x = row-in-block],
    free f = column = (J, y) [J = col-block 0..31, y = col-in-block].
    Fully contiguous 256 KB DMA per image.
  - One matmul per strip with the block-diagonal BD (16 copies of A^T) as
    the STATIONARY operand and the data as moving:
        U[(G,v), (J,y)] = sum_x A[v,x] X[(G,x), (J,y)]
    The output element O[8G+y, 8J+v] = U[(G,v),(J,y)] needs the within-block
    v<->y swap between partition and free before it can be stored with long
    contiguous runs.
  - Two DVE stream-transposes (32x32-block transpose; swaps the partition
    index mod 32 with the AP-selected within-32 free offset) perform that
    swap entirely on-chip:
      ST#1 (PSUM->SBUF, free viewed as (y, J)):
           p = (Ghi, g2, v) -> (Ghi, J);  f -> (y, g2, v)
      ST#2 (SBUF->SBUF, free viewed as (v, g2, y)):
           p = (Ghi, J) -> (Ghi, g2, y) = output row;  f -> (J, v) = column
    where G = 4*Ghi + g2.
  - Store is then a single fully-contiguous 256 KB DMA per image.
Every DMA moves >=1KB contiguous runs; loads on sync HWDGE, stores on
scalar HWDGE so the two streams ride different rings.
"""

import sys

sys.path.insert(0, "/opt/trn_rl_repo")

from contextlib import ExitStack

import numpy as np

import concourse.bass as bass  # noqa: F401
import concourse.tile as tile
from concourse import bacc, mybir
from concourse.bass_utils import run_bass_kernel_spmd

P = 8
H = W = 256
B, C = 16, 64
NCORES = 8
BPC = B // NCORES  # batches per core
IMGS = BPC * C  # images (b,c planes) per core
ROWS = IMGS * H  # dram rows per core

TRACE = False
LAST_RESULTS = None

_nc_cache = None


def _ensure_ntff_hook():
    """The agent image's antenv lacks axon_hooks; synthesize it so
    run_bass_kernel_spmd(trace=True) can capture NTFF profiles."""
    import types

    if "antenv.axon_hooks" in sys.modules:
        return
    try:
        sys.path.insert(0, "/root/.axon_site/trn_agent_boot")
        from trn_boot import _ntff_profile_via_ctypes

        hook = _ntff_profile_via_ctypes("/opt/axon/libaxon_pjrt.so")
    except Exception:
        hook = None
    mod = types.ModuleType("antenv.axon_hooks")
    mod._hook = hook
    mod.get_axon_ntff_profile_hook = lambda: mod._hook
    mod.set_axon_ntff_profile_hook = lambda h: setattr(mod, "_hook", h)
    sys.modules["antenv.axon_hooks"] = mod


def _dct_kernel(tc, o, x, bd):
    nc = tc.nc
    with ExitStack() as ctx:
        xpool = ctx.enter_context(tc.tile_pool(name="xin", bufs=6))
        z0pool = ctx.enter_context(tc.tile_pool(name="z0", bufs=8))
        z1pool = ctx.enter_context(tc.tile_pool(name="z1", bufs=8))
        z2pool = ctx.enter_context(tc.tile_pool(name="zout", bufs=6))
        cpool = ctx.enter_context(tc.tile_pool(name="const", bufs=1))
        ppool = ctx.enter_context(tc.tile_pool(name="ps", bufs=8, space="PSUM"))

        bdt = cpool.tile([128, 128], mybir.dt.float32r)
        nc.sync.dma_start(bdt[:], bd[:])

        for img in range(IMGS):
            # ---- load image (256x256) as [p = row%128, (r = row//128, c)] ----
            xt = xpool.tile([128, 2 * W], mybir.dt.float32r)
            src = x[img * H : (img + 1) * H, :].rearrange("(r p) c -> p r c", p=128)
            nc.sync.dma_start(xt[:].rearrange("p (r c) -> p r c", c=W), src)

            # ---- U[(G,v), (r,J,y)] = sum_x A[v,x] X[(G,x), (r,J,y)] ----
            # one 512-wide fp32r matmul per image (both strips), one PSUM bank
            ps = ppool.tile([128, 2 * W], mybir.dt.float32)
            nc.tensor.matmul(ps[:], bdt[:], xt[:], start=True, stop=True)

            # ---- ACT: PSUM fp32 -> SBUF bf16, reorder (r,J,y)->(r,y,J) ----
            z0 = z0pool.tile([128, 2 * W], mybir.dt.bfloat16)
            nc.scalar.activation(
                z0[:].rearrange("p (r y J) -> p r y J", r=2, y=P),
                ps[:].rearrange("p (r J y) -> p r y J", r=2, y=P),
                mybir.ActivationFunctionType.Copy,
            )

            # ---- ST#1 (bf16): p (Ghi,g2,v)->(Ghi,J); z1 addr = (g*8+v)*16
            #      + (r*8+y): both STs use <=3 free AP dims (walrus limit) ----
            z1 = z1pool.tile([128, 2 * W], mybir.dt.bfloat16)
            nc.vector.transpose(
                z1[:].rearrange("p (g v r y) -> p (r y) (g v)", g=4, v=P, r=2),
                z0[:].rearrange("p (ry J) -> p ry J", ry=16),
            )

            # ---- ST#2 (bf16): p (Ghi,J)->(Ghi,g2,y) = row; f ->(r,J,v) ----
            zt = z2pool.tile([128, 2 * W], mybir.dt.bfloat16)
            nc.vector.transpose(
                zt[:].rearrange("p (r J v) -> p v r J", J=32, v=P),
                z1[:].rearrange("p (g v r y) -> p (v r) g y", g=4, v=P, r=2),
            )

            # ---- store image: fully contiguous 256 KB ----
            dst = o[img * H : (img + 1) * H, :].rearrange("(r p) c -> p r c", p=128)
            nc.scalar.dma_start(dst, zt[:].rearrange("p (r c) -> p r c", c=W))


def _build_nc():
    nc = bacc.Bacc(
        "TRN2", target_bir_lowering=False, debug=False, num_devices=NCORES
    )
    x_ap = nc.dram_tensor(
        "x", [ROWS, W], mybir.dt.float32r, kind="ExternalInput"
    ).ap()
    bd_ap = nc.dram_tensor(
        "bd", [128, 128], mybir.dt.float32r, kind="ExternalInput"
    ).ap()
    o_ap = nc.dram_tensor(
        "o", [ROWS, W], mybir.dt.bfloat16, kind="ExternalOutput"
    ).ap()
    with tile.TileContext(nc) as tc:
        _dct_kernel(tc, o_ap, x_ap, bd_ap)
    nc.compile()
    return nc


def _make_bd(dct_basis: np.ndarray) -> np.ndarray:
    a = dct_basis.astype(np.float64) @ dct_basis.astype(np.float64)
    at = a.T.astype(np.float32)  # block[x, v] = A[v, x]
    bd = np.zeros((128, 128), dtype=np.float32)
    for g in range(16):
        bd[g * P : (g + 1) * P, g * P : (g + 1) * P] = at
    return bd


def kernel(x: np.ndarray, dct_basis: np.ndarray) -> np.ndarray:
    global _nc_cache, LAST_RESULTS
    x = np.asarray(x, dtype=np.float32)
    dct_basis = np.asarray(dct_basis, dtype=np.float32)
    assert x.shape == (B, C, H, W)

    if _nc_cache is None:
        _nc_cache = _build_nc()
    nc = _nc_cache

    bd = _make_bd(dct_basis)
    in_maps = []
    for i in range(NCORES):
        xs = np.ascontiguousarray(x[i * BPC : (i + 1) * BPC]).reshape(ROWS, W)
        in_maps.append({"x": xs, "bd": bd})

    if TRACE:
        _ensure_ntff_hook()
    try:
        res = run_bass_kernel_spmd(
            nc, in_maps, core_ids=list(range(NCORES)), trace=TRACE
        )
    except ModuleNotFoundError:
        res = run_bass_kernel_spmd(
            nc, in_maps, core_ids=list(range(NCORES)), trace=False
        )
    LAST_RESULTS = res

    out = np.empty((B, C, H, W), dtype=np.float32)
    for i in range(NCORES):
        out[i * BPC : (i + 1) * BPC] = (
            res.results[i]["o"].astype(np.float32).reshape(BPC, C, H, W)
        )
    return out
